# revision 51
# baseline (speedup 1.0000x reference)
"""Multi-head attention Trainium2 kernel (B=4, S=2048, D=1024, H=16, causal).

Sharding: 8 cores = 4 batches x 2 head-groups (8 heads each, tensor-parallel
over the QKV/out projection weights along the head dimension).

Single software-pipelined phase per core (no phase barriers):
  - All matmul operands are bf16 (PSUM accumulation stays f32), which keeps
    the full PE rate while halving DMA bytes and SBUF footprint and enabling
    the DVE 2x packed mode for the causal-mask multiplies.
  - Attention starts as soon as the minimal prefix (k/v/q of the first
    s-block's first row-block) is projected; all remaining projection and
    out-projection work is issued as PE "filler" units interleaved into the
    attention stages, whose steady state is ACT(exp)-paced.
  - Attention q-block order is 0, 2, 3, 1 so the final stages still have
    PE filler available and the kernel tail stays short.
  - scoresT[k, q] = khT.T @ qhT per head (two heads row-packed in the PE
    array); exp on ACT with the 1/sqrt(dk) scale folded in; V augmented
    with a ones column so the ctx matmul also accumulates the softmax
    denominator; normalize via DVE reciprocal + GpSimd partition_broadcast.
  - Per-core partial outputs are summed pairwise (+ bo) on the host.
"""

import numpy as np
import ml_dtypes

import concourse.bacc as bacc
import concourse.mybir as mybir
import concourse.tile as tile
from concourse.bass_utils import run_bass_kernel_spmd

B, S, D, H = 4, 2048, 1024, 16
DK = D // H          # 64
N_CORES = 8
O = 512              # head dims per core (8 heads x 64)
HPC = 8              # heads per core
SB = 512             # s-block for projections
QB = 512             # q-block for attention
KT = 128             # k tile
F32 = mybir.dt.float32
BF16 = mybir.dt.bfloat16
BF16NP = ml_dtypes.bfloat16

_CACHE = {}


def _build(s=S):
    """Build the per-core SPMD program. Returns the Bacc module."""
    nc = bacc.Bacc("TRN2", target_bir_lowering=False, debug=False,
                   num_devices=N_CORES)
    n_sc = s // 128           # 16 s chunks of 128

    xqT = nc.declare_dram_parameter("xqT", [D, s], BF16, isOutput=False)
    xkT = nc.declare_dram_parameter("xkT", [D, s], BF16, isOutput=False)
    xvT = nc.declare_dram_parameter("xvT", [D, s], BF16, isOutput=False)
    wqT = nc.declare_dram_parameter("wqT", [D, O], BF16, isOutput=False)
    wkT = nc.declare_dram_parameter("wkT", [D, O], BF16, isOutput=False)
    wvT = nc.declare_dram_parameter("wvT", [D, O], BF16, isOutput=False)
    bqd = nc.declare_dram_parameter("bq", [O], F32, isOutput=False)
    bkd = nc.declare_dram_parameter("bk", [O], F32, isOutput=False)
    bvb = nc.declare_dram_parameter("bv_bc", [128, O], F32, isOutput=False)
    wod = nc.declare_dram_parameter("woT", [O, D], BF16, isOutput=False)
    maskd = nc.declare_dram_parameter("masks", [KT, KT], BF16,
                                      isOutput=False)
    onesd = nc.declare_dram_parameter("ones8", [128, HPC], BF16,
                                      isOutput=False)
    outd = nc.declare_dram_parameter("out", [s, D], F32, isOutput=True)

    scale = float(DK) ** -0.5

    xq_r = xqT.ap().rearrange("(a p) s -> p a s", p=128)
    xk_r = xkT.ap().rearrange("(a p) s -> p a s", p=128)
    xv_r = xvT.ap().rearrange("(a p) s -> p a s", p=128)
    wq_r = wqT.ap().rearrange("(a p) o -> p a o", p=128)
    wk_r = wkT.ap().rearrange("(a p) o -> p a o", p=128)
    wv_r = wvT.ap().rearrange("(a p) o -> p a o", p=128)
    wo_r = wod.ap().rearrange("(a p) o -> p a o", p=128)

    with tile.TileContext(nc) as tc:
        with (
            tc.tile_pool(name="res", bufs=1) as res,
            tc.tile_pool(name="x0pool", bufs=1) as x0pool,
            tc.tile_pool(name="xtpool", bufs=5) as xtpool,
            tc.tile_pool(name="epool", bufs=4) as epool,
            tc.tile_pool(name="npool", bufs=3) as npool,
            tc.tile_pool(name="outpool", bufs=8) as outpool,
        ):
            psum2 = tc.alloc_tile_pool(name="psum2", bufs=2, space="PSUM")
            psum1 = tc.alloc_tile_pool(name="psum1", bufs=1, space="PSUM")

            # ---- residents ----
            qhT = [res.tile([128, s], BF16, tag=f"qhT{j}", name=f"qhT{j}")
                   for j in range(4)]
            khT = [res.tile([128, s], BF16, tag=f"khT{j}", name=f"khT{j}")
                   for j in range(4)]
            vh = [res.tile([128, HPC, DK + 1], BF16, tag=f"vh{i}",
                           name=f"vh{i}") for i in range(n_sc)]
            ctxT = [res.tile([128, s], BF16, tag=f"ctxT{j}", name=f"ctxT{j}")
                    for j in range(4)]
            wq_sb = res.tile([128, 8, O], BF16, tag="wq", name="wq")
            wk_sb = res.tile([128, 8, O], BF16, tag="wk", name="wk")
            wv_sb = res.tile([128, 8, O], BF16, tag="wv", name="wv")
            wo_sb = res.tile([128, 4, D], BF16, tag="wo", name="wo")
            bq_t = res.tile([128, O // 128], F32, tag="bq_t", name="bq_t")
            bk_t = res.tile([128, O // 128], F32, tag="bk_t", name="bk_t")
            bv_t = res.tile([128, O], F32, tag="bv_t", name="bv_t")
            ones_t = res.tile([128, HPC], BF16, tag="ones_t", name="ones_t")
            masks = res.tile([128, KT], BF16, tag="masks", name="masks")

            # ---- startup DMAs: minimal bytes first, ordered so each C0
            # prefix unit's operands arrive just before it runs. The m1-3
            # column blocks of wk/wq arrive later (their units are C1
            # fillers).
            xk0 = x0pool.tile([128, 8, SB], BF16, tag="xk0", name="xk0")
            xv0 = x0pool.tile([128, 8, SB], BF16, tag="xv0", name="xv0")
            nc.sync.dma_start(wk_sb[:, 0:4, 0:128], wk_r[:, 0:4, 0:128])
            nc.sync.dma_start(xk0[:, 0:2, :], xk_r[:, 0:2, 0:SB])
            nc.sync.dma_start(wk_sb[:, 4:8, 0:128], wk_r[:, 4:8, 0:128])
            nc.sync.dma_start(xk0[:, 2:4, :], xk_r[:, 2:4, 0:SB])
            nc.sync.dma_start(xk0[:, 4:8, :], xk_r[:, 4:8, 0:SB])
            nc.sync.dma_start(
                bk_t[:], bkd.ap().rearrange("(m p) -> p m", p=128))
            nc.sync.dma_start(wv_sb[:, :, 0:O // 2], wv_r[:, :, 0:O // 2])
            nc.sync.dma_start(bv_t[:], bvb[:, :])
            nc.sync.dma_start(ones_t[:], onesd[:, :])
            for h in range(4):
                hs = slice(h * 2, (h + 1) * 2)
                nc.sync.dma_start(xv0[:, hs, :], xv_r[:, hs, 0:SB])
            nc.sync.dma_start(masks[:], maskd[:, :])
            xq0 = xtpool.tile([128, 8, SB], BF16, tag="xt", name="xq0")
            nc.sync.dma_start(xq0[:], xq_r[:, :, 0:SB])
            nc.sync.dma_start(wq_sb[:, :, 0:128], wq_r[:, :, 0:128])
            nc.sync.dma_start(
                bq_t[:], bqd.ap().rearrange("(m p) -> p m", p=128))
            nc.sync.dma_start(wv_sb[:, :, O // 2:O], wv_r[:, :, O // 2:O])
            nc.sync.dma_start(wk_sb[:, :, 128:512], wk_r[:, :, 128:512])
            nc.sync.dma_start(wq_sb[:, :, 128:512], wq_r[:, :, 128:512])
            nc.sync.dma_start(wo_sb[:], wo_r[:, :, :])

            # ---- unit builders ----
            def dma_xt(xr, ts, nm):
                t = xtpool.tile([128, 8, SB], BF16, tag="xt", name=nm)
                nc.sync.dma_start(t[:], xr[:, :, ts * SB:(ts + 1) * SB])
                return t

            def qk_unit(w_sb, xb, bt, dest, m, ts, on_act=False):
                """One 128-row block of a q/k projection for s-block ts.
                on_act routes the bias-add through the scalar engine
                (idle during the prefix) instead of DVE."""
                ssl = slice(ts * SB, (ts + 1) * SB)
                ps = psum2.tile([128, SB], F32, tag="pp", name="ps_qk")
                for d in range(8):
                    nc.tensor.matmul(
                        ps[:], w_sb[:, d, m * 128:(m + 1) * 128],
                        xb[:, d, :], start=(d == 0), stop=(d == 7))
                if on_act:
                    nc.scalar.activation(
                        dest[m][:, ssl], ps[:],
                        mybir.ActivationFunctionType.Identity,
                        bias=bt[:, m:m + 1])
                else:
                    nc.vector.tensor_scalar_add(dest[m][:, ssl], ps[:],
                                                bt[:, m:m + 1])

            def v_unit(xb, ts, sc, half=None):
                """One 128-seq chunk of the v projection for s-block ts.
                half=0/1 projects only 4 of the 8 heads (256 wv columns),
                so the first attention pairs can start on half the wv
                bytes; half=None does all 8 heads."""
                si = ts * (SB // 128) + sc
                if half is None:
                    osl, hsl, w = slice(0, O), slice(0, HPC), O
                else:
                    osl = slice(half * (O // 2), (half + 1) * (O // 2))
                    hsl = slice(half * (HPC // 2), (half + 1) * (HPC // 2))
                    w = O // 2
                ps = psum2.tile([128, w], F32, tag="pp", name="ps_v")
                for d in range(8):
                    nc.tensor.matmul(
                        ps[:], xb[:, d, sc * 128:(sc + 1) * 128],
                        wv_sb[:, d, osl], start=(d == 0), stop=(d == 7))
                nc.vector.tensor_tensor(
                    vh[si][:, hsl, 0:DK],
                    ps[:].rearrange("p (h e) -> p h e", e=DK),
                    bv_t[:, osl].rearrange("p (h e) -> p h e", e=DK),
                    op=mybir.AluOpType.add)
                nc.vector.tensor_copy(vh[si][:, hsl, DK], ones_t[:, hsl])

            def op_half(sc, oc, tag="pp"):
                """Out-projection of one [128 q, 512 dout] half-tile."""
                osl = slice(oc * 512, (oc + 1) * 512)
                ot = outpool.tile([128, 512], F32, tag="out_t", name="ot")
                ps = psum2.tile([128, 512], F32, tag=tag, name="ps_o")
                for jw in range(4):
                    nc.tensor.matmul(
                        ps[:], ctxT[jw][:, sc * 128:(sc + 1) * 128],
                        wo_sb[:, jw, osl], start=(jw == 0), stop=(jw == 3))
                nc.vector.tensor_copy(ot[:], ps[:])
                nc.sync.dma_start(outd[sc * 128:(sc + 1) * 128, osl], ot[:])

            def attn_stage(qb, fillers, hold=0, pre=None):
                """Attention for q-block qb; pops filler units between
                k-tiles so the PE stays fed while ACT runs exp. `hold`
                units are kept back for the stage end, covering the last
                pair's normalize-chain latency. `pre[j]` units produce data
                pair j reads and MUST be issued before its first matmul
                (issue order defines Tile's dependency direction)."""
                nt = (qb + 1) * (QB // KT)
                qsl = slice(qb * QB, (qb + 1) * QB)
                n_steps = 4 * nt
                fper = max(len(fillers) - hold, 0) / n_steps if fillers else 0.0
                acc = 0.0
                for j in range(4):          # head pairs
                    for unit in (pre or {}).get(j, ()):
                        unit()
                    h0, h1 = 2 * j, 2 * j + 1
                    c0 = psum1.tile([DK + 1, QB], F32, tag="cacc0",
                                    name="c0")
                    c1 = psum1.tile([DK + 1, QB], F32, tag="cacc1",
                                    name="c1")
                    for t in range(nt):
                        ksl = slice(t * KT, (t + 1) * KT)
                        jj = t - (QB // KT) * qb
                        lo = jj * KT if jj > 0 else 0
                        qn = slice(qb * QB + lo, (qb + 1) * QB)
                        s01 = psum2.tile([128, 2, QB], F32, tag="sc01",
                                         name="s01")
                        nc.tensor.matmul(
                            s01[:, 0, lo:], khT[j][0:64, ksl],
                            qhT[j][0:64, qn], start=True, stop=True)
                        nc.tensor.matmul(
                            s01[:, 1, lo:], khT[j][64:128, ksl],
                            qhT[j][64:128, qn], start=True, stop=True,
                            tile_position=(64, 0))
                        e01 = epool.tile([128, 2, QB], BF16, tag="e01",
                                         name="e01")
                        nc.scalar.activation(
                            e01[:, :, lo:], s01[:, :, lo:],
                            mybir.ActivationFunctionType.Exp, scale=scale)
                        if jj >= 0:     # causal strip
                            nc.vector.tensor_mul(
                                e01[:, :, lo:lo + KT],
                                e01[:, :, lo:lo + KT],
                                masks[:].unsqueeze(1).broadcast_to(
                                    [128, 2, KT]))
                        nc.tensor.matmul(
                            c0[:, lo:], vh[t][:, h0, :], e01[:, 0, lo:],
                            start=(t == 0), stop=(t == nt - 1))
                        nc.tensor.matmul(
                            c1[:, lo:], vh[t][:, h1, :], e01[:, 1, lo:],
                            start=(t == 0), stop=(t == nt - 1))
                        acc += fper
                        while acc >= 1.0 and fillers:
                            fillers.pop(0)()
                            acc -= 1.0
                    # normalize by the denominator (PSUM row DK)
                    r0 = npool.tile([1, QB], F32, tag="r0", name="r0")
                    r1 = npool.tile([1, QB], F32, tag="r1", name="r1")
                    nc.vector.reciprocal(r0[:], c0[DK:DK + 1, :])
                    nc.vector.reciprocal(r1[:], c1[DK:DK + 1, :])
                    rb0 = npool.tile([64, QB], F32, tag="rb0", name="rb0")
                    rb1 = npool.tile([64, QB], F32, tag="rb1", name="rb1")
                    nc.gpsimd.partition_broadcast(rb0[:], r0[:])
                    nc.gpsimd.partition_broadcast(rb1[:], r1[:])
                    nc.vector.tensor_mul(ctxT[j][0:64, qsl], c0[0:DK, :],
                                         rb0[:])
                    nc.vector.tensor_mul(ctxT[j][64:128, qsl], c1[0:DK, :],
                                         rb1[:])
                while fillers:
                    fillers.pop(0)()

            # ---- C0 prefix: the minimum needed to start attention qb0 ----
            qk_unit(wk_sb, xk0, bk_t, khT, 0, 0, on_act=True)
            for u in range(4):
                v_unit(xv0, 0, u, half=0)
            qk_unit(wq_sb, xq0, bq_t, qhT, 0, 0, on_act=True)

            # prefetch x for s-blocks 1 and 2
            xk1 = dma_xt(xk_r, 1, "xk1")
            xv1 = dma_xt(xv_r, 1, "xv1")
            xk2 = dma_xt(xk_r, 2, "xk2")
            xv2 = dma_xt(xv_r, 2, "xv2")

            # ---- C1: attention qb0 ----
            # sb0's remaining k/q blocks lead the flow: they are DMA-free
            # (x0 is resident) and the pop schedule issues pair m's units
            # before pair m's first read (pops 2m-1, 2m land by step m+1 at
            # fper 1.375, pair m starts at step 4m).
            f = []
            for m in range(1, 4):
                f.append(lambda m=m: qk_unit(wk_sb, xk0, bk_t, khT, m, 0))
                f.append(lambda m=m: qk_unit(wq_sb, xq0, bq_t, qhT, m, 0))
                if m < 3:
                    f.append(lambda u=2 * m - 2: v_unit(xv0, 0, u, half=1))
                    f.append(lambda u=2 * m - 1: v_unit(xv0, 0, u, half=1))
            for u in range(4):
                f.append(lambda u=u: qk_unit(wk_sb, xk1, bk_t, khT, u, 1))
                f.append(lambda u=u: v_unit(xv1, 1, u))
            for u in range(4):
                f.append(lambda u=u: qk_unit(wk_sb, xk2, bk_t, khT, u, 2))
                f.append(lambda u=u: v_unit(xv2, 2, u))
            attn_stage(0, f)

            xq2 = dma_xt(xq_r, 2, "xq2")
            xk3 = dma_xt(xk_r, 3, "xk3")
            xv3 = dma_xt(xv_r, 3, "xv3")

            # ---- C2: attention qb2 (needs kv sb0-2 + q sb2) ----
            pre = {m: [lambda m=m: qk_unit(wq_sb, xq2, bq_t, qhT, m, 2)]
                   for m in range(4)}
            f = []
            for u in range(4):
                f.append(lambda u=u: qk_unit(wk_sb, xk3, bk_t, khT, u, 3))
                f.append(lambda u=u: v_unit(xv3, 3, u))
            attn_stage(2, f, hold=4, pre=pre)

            xq3 = dma_xt(xq_r, 3, "xq3")
            xq1 = dma_xt(xq_r, 1, "xq1")

            # ---- C3: attention qb3 ----
            pre = {m: [lambda m=m: qk_unit(wq_sb, xq3, bq_t, qhT, m, 3)]
                   for m in range(4)}
            f = []
            for sc in range(0, 4):
                for oc in range(2):
                    f.append(lambda sc=sc, oc=oc: op_half(sc, oc))
            for sc in range(8, 10):
                for oc in range(2):
                    f.append(lambda sc=sc, oc=oc: op_half(sc, oc))
            attn_stage(3, f, hold=4, pre=pre)

            # ---- C4: attention qb1 ----
            pre = {m: [lambda m=m: qk_unit(wq_sb, xq1, bq_t, qhT, m, 1)]
                   for m in range(4)}
            f = []
            for sc in range(10, 12):
                for oc in range(2):
                    f.append(lambda sc=sc, oc=oc: op_half(sc, oc))
            for sc in range(12, 16):
                for oc in range(2):
                    f.append(lambda sc=sc, oc=oc: op_half(sc, oc))
            attn_stage(1, f, hold=3, pre=pre)

            # ---- C5: out-projection of qb1 (sc01 ring is idle now, so
            # alternate tags to get four half-units in flight at once) ----
            for sc in range(4, 8):
                for oc in range(2):
                    op_half(sc, oc, tag="pp" if oc == 0 else "sc01")

            psum1.release()
            psum2.release()

    nc.compile()
    return nc


def _get_nc(s=S):
    if s not in _CACHE:
        _CACHE[s] = _build(s)
    return _CACHE[s]


def _make_masks(s=S):
    # triangular strip: valid iff local q index >= local k index
    m = np.zeros((KT, KT), np.float32)
    for kk in range(KT):
        m[kk, kk:] = 1.0
    return m


def make_in_maps(q, k, v, Wq, bq, Wk, bk, Wv, bv, Wo, s=S):
    masks = _make_masks(s).astype(BF16NP)
    in_maps = []
    for c in range(N_CORES):
        b, g = c // 2, c % 2
        gsl = slice(g * O, (g + 1) * O)
        in_maps.append({
            "xqT": np.ascontiguousarray(q[b].T).astype(BF16NP),
            "xkT": np.ascontiguousarray(k[b].T).astype(BF16NP),
            "xvT": np.ascontiguousarray(v[b].T).astype(BF16NP),
            "wqT": np.ascontiguousarray(Wq[gsl, :].T).astype(BF16NP),
            "wkT": np.ascontiguousarray(Wk[gsl, :].T).astype(BF16NP),
            "wvT": np.ascontiguousarray(Wv[gsl, :].T).astype(BF16NP),
            "bq": np.ascontiguousarray(bq[gsl]),
            "bk": np.ascontiguousarray(bk[gsl]),
            "bv_bc": np.ascontiguousarray(
                np.broadcast_to(bv[gsl][None, :], (128, O))),
            "woT": np.ascontiguousarray(Wo[:, gsl].T).astype(BF16NP),
            "ones8": np.ones((128, HPC), BF16NP),
            "masks": masks,
        })
    return in_maps


def kernel(q, k, v, mask, Wq, bq, Wk, bk, Wv, bv, Wo, bo):
    q = np.asarray(q, np.float32)
    k = np.asarray(k, np.float32)
    v = np.asarray(v, np.float32)
    nc = _get_nc(S)
    in_maps = make_in_maps(q, k, v,
                           np.asarray(Wq, np.float32), np.asarray(bq, np.float32),
                           np.asarray(Wk, np.float32), np.asarray(bk, np.float32),
                           np.asarray(Wv, np.float32), np.asarray(bv, np.float32),
                           np.asarray(Wo, np.float32), S)
    res = run_bass_kernel_spmd(nc, in_maps, list(range(N_CORES)))
    bo = np.asarray(bo, np.float32)
    out = np.empty((B, S, D), np.float32)
    for b in range(B):
        out[b] = res.results[2 * b]["out"] + res.results[2 * b + 1]["out"] + bo
    return out


# revision 62
# speedup vs baseline: 1.0155x; 1.0155x over previous
"""Multi-head attention Trainium2 kernel (B=4, S=2048, D=1024, H=16, causal).

Sharding: 8 cores = 4 batches x 2 head-groups (8 heads each, tensor-parallel
over the QKV/out projection weights along the head dimension).

Single software-pipelined phase per core (no phase barriers):
  - All matmul operands are bf16 (PSUM accumulation stays f32), which keeps
    the full PE rate while halving DMA bytes and SBUF footprint and enabling
    the DVE 2x packed mode for the causal-mask multiplies.
  - Attention starts as soon as the minimal prefix (k/v/q of the first
    s-block's first row-block) is projected; all remaining projection and
    out-projection work is issued as PE "filler" units interleaved into the
    attention stages, whose steady state is ACT(exp)-paced.
  - Attention q-block order is 0, 2, 3, 1 so the final stages still have
    PE filler available and the kernel tail stays short.
  - scoresT[k, q] = khT.T @ qhT per head (two heads row-packed in the PE
    array); exp on ACT with the 1/sqrt(dk) scale folded in; V augmented
    with a ones column so the ctx matmul also accumulates the softmax
    denominator; normalize via DVE reciprocal + GpSimd partition_broadcast.
  - Per-core partial outputs are summed pairwise (+ bo) on the host.
"""

import numpy as np
import ml_dtypes

import concourse.bacc as bacc
import concourse.mybir as mybir
import concourse.tile as tile
from concourse.bass_utils import run_bass_kernel_spmd

B, S, D, H = 4, 2048, 1024, 16
DK = D // H          # 64
N_CORES = 8
O = 512              # head dims per core (8 heads x 64)
HPC = 8              # heads per core
SB = 512             # s-block for projections
QB = 512             # q-block for attention
KT = 128             # k tile
F32 = mybir.dt.float32
BF16 = mybir.dt.bfloat16
BF16NP = ml_dtypes.bfloat16

_CACHE = {}


def _build(s=S):
    """Build the per-core SPMD program. Returns the Bacc module."""
    nc = bacc.Bacc("TRN2", target_bir_lowering=False, debug=False,
                   num_devices=N_CORES)
    n_sc = s // 128           # 16 s chunks of 128

    xqT = nc.declare_dram_parameter("xqT", [D, s], BF16, isOutput=False)
    xkT = nc.declare_dram_parameter("xkT", [D, s], BF16, isOutput=False)
    xvT = nc.declare_dram_parameter("xvT", [D, s], BF16, isOutput=False)
    wqT = nc.declare_dram_parameter("wqT", [D, O], BF16, isOutput=False)
    wkT = nc.declare_dram_parameter("wkT", [D, O], BF16, isOutput=False)
    wvT = nc.declare_dram_parameter("wvT", [D, O], BF16, isOutput=False)
    bqd = nc.declare_dram_parameter("bq", [O], F32, isOutput=False)
    bvb = nc.declare_dram_parameter("bv_row", [1, O], F32, isOutput=False)
    wod = nc.declare_dram_parameter("woT", [O, D], BF16, isOutput=False)
    maskd = nc.declare_dram_parameter("masks", [KT, KT], BF16,
                                      isOutput=False)
    onesd = nc.declare_dram_parameter("ones8", [128, HPC], BF16,
                                      isOutput=False)
    outd = nc.declare_dram_parameter("out", [s, D], F32, isOutput=True)

    scale = float(DK) ** -0.5

    xq_r = xqT.ap().rearrange("(a p) s -> p a s", p=128)
    xk_r = xkT.ap().rearrange("(a p) s -> p a s", p=128)
    xv_r = xvT.ap().rearrange("(a p) s -> p a s", p=128)
    wq_r = wqT.ap().rearrange("(a p) o -> p a o", p=128)
    wk_r = wkT.ap().rearrange("(a p) o -> p a o", p=128)
    wv_r = wvT.ap().rearrange("(a p) o -> p a o", p=128)
    wo_r = wod.ap().rearrange("(a p) o -> p a o", p=128)

    with tile.TileContext(nc) as tc:
        with (
            tc.tile_pool(name="res", bufs=1) as res,
            tc.tile_pool(name="x0pool", bufs=1) as x0pool,
            tc.tile_pool(name="xtpool", bufs=5) as xtpool,
            tc.tile_pool(name="epool", bufs=4) as epool,
            tc.tile_pool(name="npool", bufs=3) as npool,
            tc.tile_pool(name="outpool", bufs=8) as outpool,
        ):
            psum2 = tc.alloc_tile_pool(name="psum2", bufs=2, space="PSUM")
            psum1 = tc.alloc_tile_pool(name="psum1", bufs=1, space="PSUM")

            # ---- residents ----
            qhT = [res.tile([128, s], BF16, tag=f"qhT{j}", name=f"qhT{j}")
                   for j in range(4)]
            khT = [res.tile([128, s], BF16, tag=f"khT{j}", name=f"khT{j}")
                   for j in range(4)]
            vh = [res.tile([128, HPC, DK + 1], BF16, tag=f"vh{i}",
                           name=f"vh{i}") for i in range(n_sc)]
            ctxT = [res.tile([128, s], BF16, tag=f"ctxT{j}", name=f"ctxT{j}")
                    for j in range(4)]
            wq_sb = res.tile([128, 8, O], BF16, tag="wq", name="wq")
            wk_sb = res.tile([128, 8, O], BF16, tag="wk", name="wk")
            wv_sb = res.tile([128, 8, O], BF16, tag="wv", name="wv")
            wo_sb = res.tile([128, 4, D], BF16, tag="wo", name="wo")
            bq_t = res.tile([128, O // 128], F32, tag="bq_t", name="bq_t")
            bv_t = res.tile([128, O], F32, tag="bv_t", name="bv_t")
            bv_row = res.tile([1, O], F32, tag="bv_row", name="bv_row")
            ones_t = res.tile([128, HPC], BF16, tag="ones_t", name="ones_t")
            masks = res.tile([128, KT], BF16, tag="masks", name="masks")

            # ---- startup DMAs: minimal bytes first, ordered so each C0
            # prefix unit's operands arrive just before it runs. The m1-3
            # column blocks of wk/wq arrive later (their units are C1
            # fillers).
            xk0 = x0pool.tile([128, 8, SB], BF16, tag="xk0", name="xk0")
            xv0 = x0pool.tile([128, 8, SB], BF16, tag="xv0", name="xv0")
            nc.sync.dma_start(wk_sb[:, 0:4, 0:128], wk_r[:, 0:4, 0:128])
            nc.sync.dma_start(xk0[:, 0:2, :], xk_r[:, 0:2, 0:SB])
            nc.sync.dma_start(wk_sb[:, 4:8, 0:128], wk_r[:, 4:8, 0:128])
            nc.sync.dma_start(xk0[:, 2:4, :], xk_r[:, 2:4, 0:SB])
            nc.sync.dma_start(xk0[:, 4:6, :], xk_r[:, 4:6, 0:SB])
            nc.sync.dma_start(xk0[:, 6:8, :], xk_r[:, 6:8, 0:SB])
            for h in range(2):
                hs = slice(h * 4, (h + 1) * 4)
                nc.sync.dma_start(wv_sb[:, hs, :], wv_r[:, hs, :])
            nc.sync.dma_start(bv_row[:], bvb[:, :])
            nc.gpsimd.partition_broadcast(bv_t[:], bv_row[:])
            nc.vector.memset(ones_t[:], 1.0)
            for h in range(4):
                hs = slice(h * 2, (h + 1) * 2)
                nc.sync.dma_start(xv0[:, hs, :], xv_r[:, hs, 0:SB])
            nc.sync.dma_start(masks[:], maskd[:, :])
            xq0 = xtpool.tile([128, 8, SB], BF16, tag="xt", name="xq0")
            nc.sync.dma_start(wq_sb[:, 0:4, 0:128], wq_r[:, 0:4, 0:128])
            nc.sync.dma_start(xq0[:, 0:2, :], xq_r[:, 0:2, 0:SB])
            nc.sync.dma_start(wq_sb[:, 4:8, 0:128], wq_r[:, 4:8, 0:128])
            nc.sync.dma_start(xq0[:, 2:4, :], xq_r[:, 2:4, 0:SB])
            nc.sync.dma_start(xq0[:, 4:8, :], xq_r[:, 4:8, 0:SB])
            nc.sync.dma_start(
                bq_t[:], bqd.ap().rearrange("(m p) -> p m", p=128))
            nc.sync.dma_start(wk_sb[:, :, 128:512], wk_r[:, :, 128:512])
            nc.sync.dma_start(wq_sb[:, :, 128:512], wq_r[:, :, 128:512])
            nc.sync.dma_start(wo_sb[:], wo_r[:, :, :])

            # ---- unit builders ----
            def dma_xt(xr, ts, nm):
                t = xtpool.tile([128, 8, SB], BF16, tag="xt", name=nm)
                nc.sync.dma_start(t[:], xr[:, :, ts * SB:(ts + 1) * SB])
                return t

            def qk_unit(w_sb, xb, bt, dest, m, ts, on_act=False):
                """One 128-row block of a q/k projection for s-block ts.
                bt=None skips the bias (k needs none: softmax over k is
                invariant to the per-query constant qh.bk). on_act routes
                the epilogue through the scalar engine (idle during the
                prefix) instead of DVE."""
                ssl = slice(ts * SB, (ts + 1) * SB)
                ps = psum2.tile([128, SB], F32, tag="pp", name="ps_qk")
                for d in range(8):
                    nc.tensor.matmul(
                        ps[:], w_sb[:, d, m * 128:(m + 1) * 128],
                        xb[:, d, :], start=(d == 0), stop=(d == 7))
                if bt is None:
                    if on_act:
                        nc.scalar.copy(dest[m][:, ssl], ps[:])
                    else:
                        nc.vector.tensor_copy(dest[m][:, ssl], ps[:])
                elif on_act:
                    nc.scalar.activation(
                        dest[m][:, ssl], ps[:],
                        mybir.ActivationFunctionType.Identity,
                        bias=bt[:, m:m + 1])
                else:
                    nc.vector.tensor_scalar_add(dest[m][:, ssl], ps[:],
                                                bt[:, m:m + 1])

            def v_unit(xb, ts, sc):
                """One 128-seq chunk of the v projection for s-block ts."""
                si = ts * (SB // 128) + sc
                ps = psum2.tile([128, O], F32, tag="pp", name="ps_v")
                for d in range(8):
                    nc.tensor.matmul(
                        ps[:], xb[:, d, sc * 128:(sc + 1) * 128],
                        wv_sb[:, d, :], start=(d == 0), stop=(d == 7))
                nc.vector.tensor_tensor(
                    vh[si][:, :, 0:DK],
                    ps[:].rearrange("p (h e) -> p h e", e=DK),
                    bv_t[:].rearrange("p (h e) -> p h e", e=DK),
                    op=mybir.AluOpType.add)
                nc.vector.tensor_copy(vh[si][:, :, DK], ones_t[:])

            def op_half(sc, oc, tag="pp"):
                """Out-projection of one [128 q, 512 dout] half-tile."""
                osl = slice(oc * 512, (oc + 1) * 512)
                ot = outpool.tile([128, 512], F32, tag="out_t", name="ot")
                ps = psum2.tile([128, 512], F32, tag=tag, name="ps_o")
                for jw in range(4):
                    nc.tensor.matmul(
                        ps[:], ctxT[jw][:, sc * 128:(sc + 1) * 128],
                        wo_sb[:, jw, osl], start=(jw == 0), stop=(jw == 3))
                nc.vector.tensor_copy(ot[:], ps[:])
                nc.sync.dma_start(outd[sc * 128:(sc + 1) * 128, osl], ot[:])

            def attn_stage(qb, fillers, hold=0, pre=None):
                """Attention for q-block qb; pops filler units between
                k-tiles so the PE stays fed while ACT runs exp. `hold`
                units are kept back for the stage end, covering the last
                pair's normalize-chain latency. `pre[j]` units produce data
                pair j reads and MUST be issued before its first matmul
                (issue order defines Tile's dependency direction)."""
                nt = (qb + 1) * (QB // KT)
                qsl = slice(qb * QB, (qb + 1) * QB)
                n_steps = 4 * nt
                fper = max(len(fillers) - hold, 0) / n_steps if fillers else 0.0
                acc = 0.0
                for j in range(4):          # head pairs
                    for unit in (pre or {}).get(j, ()):
                        unit()
                    h0, h1 = 2 * j, 2 * j + 1
                    c0 = psum1.tile([DK + 1, QB], F32, tag="cacc0",
                                    name="c0")
                    c1 = psum1.tile([DK + 1, QB], F32, tag="cacc1",
                                    name="c1")
                    for t in range(nt):
                        ksl = slice(t * KT, (t + 1) * KT)
                        jj = t - (QB // KT) * qb
                        lo = jj * KT if jj > 0 else 0
                        qn = slice(qb * QB + lo, (qb + 1) * QB)
                        s01 = psum2.tile([128, 2, QB], F32, tag="sc01",
                                         name="s01")
                        nc.tensor.matmul(
                            s01[:, 0, lo:], khT[j][0:64, ksl],
                            qhT[j][0:64, qn], start=True, stop=True)
                        nc.tensor.matmul(
                            s01[:, 1, lo:], khT[j][64:128, ksl],
                            qhT[j][64:128, qn], start=True, stop=True,
                            tile_position=(64, 0))
                        e01 = epool.tile([128, 2, QB], BF16, tag="e01",
                                         name="e01")
                        nc.scalar.activation(
                            e01[:, :, lo:], s01[:, :, lo:],
                            mybir.ActivationFunctionType.Exp, scale=scale)
                        if jj >= 0:     # causal strip
                            nc.vector.tensor_mul(
                                e01[:, :, lo:lo + KT],
                                e01[:, :, lo:lo + KT],
                                masks[:].unsqueeze(1).broadcast_to(
                                    [128, 2, KT]))
                        nc.tensor.matmul(
                            c0[:, lo:], vh[t][:, h0, :], e01[:, 0, lo:],
                            start=(t == 0), stop=(t == nt - 1))
                        nc.tensor.matmul(
                            c1[:, lo:], vh[t][:, h1, :], e01[:, 1, lo:],
                            start=(t == 0), stop=(t == nt - 1))
                        acc += fper
                        while acc >= 1.0 and fillers:
                            fillers.pop(0)()
                            acc -= 1.0
                    # normalize by the denominator (PSUM row DK)
                    r0 = npool.tile([1, QB], F32, tag="r0", name="r0")
                    r1 = npool.tile([1, QB], F32, tag="r1", name="r1")
                    nc.vector.reciprocal(r0[:], c0[DK:DK + 1, :])
                    nc.vector.reciprocal(r1[:], c1[DK:DK + 1, :])
                    rb0 = npool.tile([64, QB], F32, tag="rb0", name="rb0")
                    rb1 = npool.tile([64, QB], F32, tag="rb1", name="rb1")
                    nc.gpsimd.partition_broadcast(rb0[:], r0[:])
                    nc.gpsimd.partition_broadcast(rb1[:], r1[:])
                    nc.vector.tensor_mul(ctxT[j][0:64, qsl], c0[0:DK, :],
                                         rb0[:])
                    nc.vector.tensor_mul(ctxT[j][64:128, qsl], c1[0:DK, :],
                                         rb1[:])
                while fillers:
                    fillers.pop(0)()

            # ---- C0 prefix: the minimum needed to start attention qb0 ----
            qk_unit(wk_sb, xk0, None, khT, 0, 0, on_act=True)
            for u in range(4):
                v_unit(xv0, 0, u)
            qk_unit(wq_sb, xq0, bq_t, qhT, 0, 0, on_act=True)

            # prefetch x for s-blocks 1 and 2
            xk1 = dma_xt(xk_r, 1, "xk1")
            xv1 = dma_xt(xv_r, 1, "xv1")
            xk2 = dma_xt(xk_r, 2, "xk2")
            xv2 = dma_xt(xv_r, 2, "xv2")

            # ---- C1: attention qb0 ----
            # sb0's remaining k/q blocks lead the flow: they are DMA-free
            # (x0 is resident) and the pop schedule issues pair m's units
            # before pair m's first read (pops 2m-1, 2m land by step m+1 at
            # fper 1.375, pair m starts at step 4m).
            f = []
            for m in range(1, 4):
                f.append(lambda m=m: qk_unit(wk_sb, xk0, None, khT, m, 0))
                f.append(lambda m=m: qk_unit(wq_sb, xq0, bq_t, qhT, m, 0))
            for u in range(4):
                f.append(lambda u=u: qk_unit(wk_sb, xk1, None, khT, u, 1))
                f.append(lambda u=u: v_unit(xv1, 1, u))
            for u in range(4):
                f.append(lambda u=u: qk_unit(wk_sb, xk2, None, khT, u, 2))
                f.append(lambda u=u: v_unit(xv2, 2, u))
            attn_stage(0, f)

            xq2 = dma_xt(xq_r, 2, "xq2")
            xk3 = dma_xt(xk_r, 3, "xk3")
            xv3 = dma_xt(xv_r, 3, "xv3")

            # ---- C2: attention qb2 (needs kv sb0-2 + q sb2) ----
            pre = {m: [lambda m=m: qk_unit(wq_sb, xq2, bq_t, qhT, m, 2)]
                   for m in range(4)}
            f = []
            for u in range(4):
                f.append(lambda u=u: qk_unit(wk_sb, xk3, None, khT, u, 3))
                f.append(lambda u=u: v_unit(xv3, 3, u))
            attn_stage(2, f, hold=4, pre=pre)

            xq3 = dma_xt(xq_r, 3, "xq3")
            xq1 = dma_xt(xq_r, 1, "xq1")

            # ---- C3: attention qb3 ----
            pre = {m: [lambda m=m: qk_unit(wq_sb, xq3, bq_t, qhT, m, 3)]
                   for m in range(4)}
            f = []
            for sc in range(0, 4):
                for oc in range(2):
                    f.append(lambda sc=sc, oc=oc: op_half(sc, oc))
            for sc in range(8, 10):
                for oc in range(2):
                    f.append(lambda sc=sc, oc=oc: op_half(sc, oc))
            attn_stage(3, f, hold=4, pre=pre)

            # ---- C4: attention qb1 ----
            pre = {m: [lambda m=m: qk_unit(wq_sb, xq1, bq_t, qhT, m, 1)]
                   for m in range(4)}
            f = []
            for sc in range(10, 12):
                for oc in range(2):
                    f.append(lambda sc=sc, oc=oc: op_half(sc, oc))
            for sc in range(12, 16):
                for oc in range(2):
                    f.append(lambda sc=sc, oc=oc: op_half(sc, oc))
            attn_stage(1, f, hold=3, pre=pre)

            # ---- C5: out-projection of qb1 (sc01 ring is idle now, so
            # alternate tags to get four half-units in flight at once) ----
            for sc in range(4, 8):
                for oc in range(2):
                    op_half(sc, oc, tag="pp" if oc == 0 else "sc01")

            psum1.release()
            psum2.release()

    nc.compile()
    return nc


def _get_nc(s=S):
    if s not in _CACHE:
        _CACHE[s] = _build(s)
    return _CACHE[s]


def _make_masks(s=S):
    # triangular strip: valid iff local q index >= local k index
    m = np.zeros((KT, KT), np.float32)
    for kk in range(KT):
        m[kk, kk:] = 1.0
    return m


def make_in_maps(q, k, v, Wq, bq, Wk, bk, Wv, bv, Wo, s=S):
    masks = _make_masks(s).astype(BF16NP)
    in_maps = []
    for c in range(N_CORES):
        b, g = c // 2, c % 2
        gsl = slice(g * O, (g + 1) * O)
        in_maps.append({
            "xqT": np.ascontiguousarray(q[b].T).astype(BF16NP),
            "xkT": np.ascontiguousarray(k[b].T).astype(BF16NP),
            "xvT": np.ascontiguousarray(v[b].T).astype(BF16NP),
            "wqT": np.ascontiguousarray(Wq[gsl, :].T).astype(BF16NP),
            "wkT": np.ascontiguousarray(Wk[gsl, :].T).astype(BF16NP),
            "wvT": np.ascontiguousarray(Wv[gsl, :].T).astype(BF16NP),
            "bq": np.ascontiguousarray(bq[gsl]),
            "bv_row": np.ascontiguousarray(bv[gsl][None, :]),
            "woT": np.ascontiguousarray(Wo[:, gsl].T).astype(BF16NP),
            "ones8": np.ones((128, HPC), BF16NP),
            "masks": masks,
        })
    return in_maps


def kernel(q, k, v, mask, Wq, bq, Wk, bk, Wv, bv, Wo, bo):
    q = np.asarray(q, np.float32)
    k = np.asarray(k, np.float32)
    v = np.asarray(v, np.float32)
    nc = _get_nc(S)
    in_maps = make_in_maps(q, k, v,
                           np.asarray(Wq, np.float32), np.asarray(bq, np.float32),
                           np.asarray(Wk, np.float32), np.asarray(bk, np.float32),
                           np.asarray(Wv, np.float32), np.asarray(bv, np.float32),
                           np.asarray(Wo, np.float32), S)
    res = run_bass_kernel_spmd(nc, in_maps, list(range(N_CORES)))
    bo = np.asarray(bo, np.float32)
    out = np.empty((B, S, D), np.float32)
    for b in range(B):
        out[b] = res.results[2 * b]["out"] + res.results[2 * b + 1]["out"] + bo
    return out
